# revision 23
# baseline (speedup 1.0000x reference)
"""DUQ RBF head kernel for Trainium2 (8 NeuronCores, batch-parallel).

Computes out[b,c,h,w] = exp(gamma * mean_e (einsum('bfhw,ecf', x, W) - m/N)^2)
for features [8,512,128,128], weights [16,64,512], m [16,64], N [64].

Strategy: data-parallel over batch (1 image per core). Per core, one big
matmul [ec=1024, f=512] @ [f=512, pix=16384] in float16 (same PE rate as
fp32r but with double-buffered 1-cycle/row weight loads), tiled as 8
ec-chunks x pixel-tiles of 1024 (two 512-wide PSUM bank halves per chunk)
x 4 K-chunks. Centroid subtraction folds into the per-partition bias of
an ACT Square epilogue writing fp16; DVE accumulates ec-chunks in fp16
(2x mode), final ACT Exp. Weights are host-packed so each SBUF partition
reads one contiguous 8KB line (256B lines were the startup bottleneck).
PE warmup matmuls absorb the p-state ramp while tile-0 x streams in over
two DMA queues; a dummy Square/Exp hoists the ACT table load early. The
scalar queue carries no DMAs (blocked triggers would stall the Squares).
The last pixel tile runs as 512+256+256 px slices to shorten the
end-of-kernel drain chain; output DMA rides the gpsimd queue.
"""

import numpy as np

import concourse.bacc as bacc_mod
import concourse.mybir as mybir
import concourse.tile as tile
from concourse.bass_utils import run_bass_kernel_spmd

dt = mybir.dt
Act = mybir.ActivationFunctionType

B, F, H, W = 8, 512, 128, 128
E, C = 16, 64
PIX = H * W          # 16384 pixels per image
NT = 1024            # pixel tile (2 psum banks)
HT = 512             # matmul moving-dim half (1 psum bank)
MCH = (E * C) // 128  # 8 ec-chunks of 128 partitions
KCH = F // 128        # 4 contraction chunks
LENGTH_SCALE = 0.1
GAMMA = -1.0 / (2.0 * LENGTH_SCALE**2)   # -50.0
EXP_SCALE = GAMMA / E                    # -3.125
WARMUP_MM = 6        # scratch matmuls to hold the PE p-state ramp

# 15 full tiles of 1024 px, then 512+256+256 (progressively shorter
# drain chain at the end of the kernel)
TILES = [(i * NT, NT) for i in range(PIX // NT - 1)]
TILES += [(PIX - NT, HT), (PIX - HT, 256), (PIX - 256, 256)]


def _build():
    nc = bacc_mod.Bacc(None)
    feat_d = nc.declare_dram_parameter("feat", [F, PIX], dt.float16, isOutput=False)
    # host-packed: wall[p, (m*KCH+k)*128 + j] = W[k*128+p -> f][m*128+j -> ec]
    wall_d = nc.declare_dram_parameter(
        "wall", [128, MCH * KCH * 128], dt.float16, isOutput=False
    )
    negc_d = nc.declare_dram_parameter("negc", [128, MCH], dt.float32, isOutput=False)
    out_d = nc.declare_dram_parameter("out", [C, PIX], dt.float32, isOutput=True)

    feat_k = feat_d.rearrange("(k p) x -> p k x", k=KCH)
    x_queues = [nc.sync, nc.gpsimd, nc.scalar, nc.gpsimd]

    with tile.TileContext(nc) as tc:
        with (
            tc.tile_pool(name="singles", bufs=1) as singles,
            tc.tile_pool(name="xin", bufs=8) as xin,
            tc.tile_pool(name="sqp", bufs=3) as sqp,
            tc.tile_pool(name="accp", bufs=2) as accp,
            tc.tile_pool(name="outp", bufs=4) as outp,
            tc.tile_pool(name="ps", bufs=4, space="PSUM") as ps,
        ):
            # Startup choreography. Needed ASAP: ws chunk 0 + all four x
            # chunks of tile 0, each on its own completion semaphore.
            # scalar: ws0, x2 (both done long before the first real Square);
            # sync: x0 then per-chunk ws1..7, negc, and all steady-state x;
            # gpsimd: x1, x3 then only output DMAs (fast end-of-kernel
            # drain). Warmup matmuls hold the PE p-state ramp meanwhile and
            # a dummy Square+Exp pulls the ACT table load early.
            ws_all = singles.tile([128, MCH * KCH * 128], dt.float16, tag="ws")
            wchunk = KCH * 128  # columns per ec-chunk
            nc.scalar.dma_start(
                out=ws_all[:, 0:wchunk], in_=wall_d[:, 0:wchunk]
            )
            xt0 = []
            for k in range(KCH):
                xtk = xin.tile([128, NT], dt.float16, tag=f"x{k}")
                x_queues[k].dma_start(out=xtk, in_=feat_k[:, k, 0:NT])
                xt0.append(xtk)

            wu_w = singles.tile([128, 16], dt.float16, tag="wu_w")
            wu_x = singles.tile([128, HT], dt.float16, tag="wu_x")
            nc.vector.memset(wu_w, 0.0)
            nc.vector.memset(wu_x, 0.0)
            wu_act = singles.tile([128, 1], dt.float32, tag="wu_act")
            nc.scalar.activation(
                out=wu_act, in_=wu_x[:, 0:1], func=Act.Square, bias=0.0, scale=1.0
            )
            wu_act2 = singles.tile([128, 1], dt.float32, tag="wu_act2")
            nc.scalar.activation(
                out=wu_act2, in_=wu_x[:, 0:1], func=Act.Exp, bias=0.0, scale=1.0
            )
            wu_ps = ps.tile([128, NT], dt.float32, tag="mm")
            for i in range(WARMUP_MM):
                nc.tensor.matmul(
                    out=wu_ps[0:16, 0:HT], lhsT=wu_w, rhs=wu_x,
                    start=(i == 0), stop=(i == WARMUP_MM - 1),
                )

            # per-chunk weight DMAs: each m-chunk's matmuls gate on their own
            # completion semaphore instead of one bulk transfer
            for m in range(1, MCH):
                nc.sync.dma_start(
                    out=ws_all[:, m * wchunk : (m + 1) * wchunk],
                    in_=wall_d[:, m * wchunk : (m + 1) * wchunk],
                )
            negc_sb = singles.tile([128, MCH], dt.float32, tag="negc")
            nc.sync.dma_start(out=negc_sb, in_=negc_d[:, :])

            def wsl(m, k):
                c0 = (m * KCH + k) * 128
                return ws_all[:, c0 : c0 + 128]

            for t, (p0, sz) in enumerate(TILES):
                px = slice(p0, p0 + sz)
                if t == 0:
                    xt = xt0
                else:
                    xt = []
                    for k in range(KCH):
                        xtk = xin.tile([128, NT], dt.float16, tag=f"x{k}")
                        nc.sync.dma_start(out=xtk[:, 0:sz], in_=feat_k[:, k, px])
                        xt.append(xtk)

                nh = (sz + HT - 1) // HT
                acc = accp.tile([128, NT], dt.float16, tag="acc")
                for m in range(MCH):
                    pst = ps.tile([128, NT], dt.float32, tag="mm")
                    for h in range(nh):
                        hs = slice(h * HT, min((h + 1) * HT, sz))
                        for k in range(KCH):
                            nc.tensor.matmul(
                                out=pst[:, hs], lhsT=wsl(m, k),
                                rhs=xt[k][:, hs],
                                start=(k == 0), stop=(k == KCH - 1),
                            )
                    if m == 0:
                        nc.scalar.activation(
                            out=acc[:, 0:sz], in_=pst[:, 0:sz], func=Act.Square,
                            bias=negc_sb[:, 0:1], scale=1.0,
                        )
                    else:
                        sq = sqp.tile([128, NT], dt.float16, tag="sq")
                        nc.scalar.activation(
                            out=sq[:, 0:sz], in_=pst[:, 0:sz], func=Act.Square,
                            bias=negc_sb[:, m : m + 1], scale=1.0,
                        )
                        nc.vector.tensor_add(
                            out=acc[:, 0:sz], in0=acc[:, 0:sz], in1=sq[:, 0:sz]
                        )

                tmp = outp.tile([64, NT], dt.float16, tag="tmp")
                nc.vector.tensor_copy(out=tmp[:, 0:sz], in_=acc[64:128, 0:sz])
                hc = outp.tile([64, NT], dt.float16, tag="hc")
                nc.vector.tensor_add(
                    out=hc[:, 0:sz], in0=acc[0:64, 0:sz], in1=tmp[:, 0:sz]
                )
                eo = outp.tile([64, NT], dt.float32, tag="eo")
                nc.scalar.activation(
                    out=eo[:, 0:sz], in_=hc[:, 0:sz], func=Act.Exp,
                    bias=0.0, scale=EXP_SCALE,
                )
                nc.gpsimd.dma_start(out=out_d[:, px], in_=eo[:, 0:sz])

    nc.finalize()
    return nc


_NC_CACHE = {}


def _get_nc():
    if "nc" not in _NC_CACHE:
        _NC_CACHE["nc"] = _build()
    return _NC_CACHE["nc"]


def _prep_inputs(features, weights, m, N):
    # wt[f, e*64+c] = weights[e, c, f]; pack so SBUF partition p reads one
    # contiguous line: wall[p, (m*KCH+k)*128+j] = wt[k*128+p, m*128+j]
    wt = (
        weights.astype(np.float32).transpose(2, 0, 1).reshape(F, E * C)
    ).astype(np.float16)
    wall = np.ascontiguousarray(
        wt.reshape(KCH, 128, MCH, 128).transpose(1, 2, 0, 3).reshape(128, -1)
    )
    cent = (m.astype(np.float32) / N.astype(np.float32)[None, :]).reshape(-1)  # [ec]
    negc = np.ascontiguousarray(-cent.reshape(MCH, 128).T)  # [128, MCH]
    feats = np.ascontiguousarray(
        features.astype(np.float32).reshape(B, F, PIX).astype(np.float16)
    )
    return [{"feat": feats[i], "wall": wall, "negc": negc} for i in range(B)]


def run_spmd(features, weights, m, N, trace=False):
    in_maps = _prep_inputs(features, weights, m, N)
    res = run_bass_kernel_spmd(_get_nc(), in_maps, list(range(B)), trace=trace)
    out = np.stack([res.results[i]["out"] for i in range(B)])  # [B, C, PIX]
    return out.reshape(B, C, H, W).astype(np.float32), res


def kernel(features, weights, m, N):
    out, _ = run_spmd(features, weights, m, N, trace=False)
    return out
